# revision 83
# baseline (speedup 1.0000x reference)
"""CrossAttentionBlock kernel for 8 Trainium2 NeuronCores.

Sharding: 16 (batch, head) pairs -> 8 cores, each core owns one batch b and
two heads (2*hp, 2*hp+1).  Per core:
  qT/kT = (Wq/Wk slice)^T-projection of condition[b]   [128=2*64 d, 4096 t]
  v     = x[b] @ Wv slice                               [4096 j, 128 dv]
  S^T   = kT^T-slices @ qT  (per head, row-packed on the PE)
  P     = exp(S^T - 8), one full [128,1024] tile per chunk, alternating
          engines (FD=1024 per op amortizes the per-instruction
          overhead): ScalarE exact exp on 20/32 chunks, DVE Schraudolph
          (bf16 bits = round(S*128*log2e + B)) on 12/32 (n%8 in 2,5,7)
  out^T = v^T @ P^T  accumulated over j (col-packed 2 heads)
  Z     = ones^T @ P^T accumulated over j: PE ones-matmul pair on 20/32
          chunks, DVE bf16 accumulator on 12/32 (folded in at block end)
  final = (out^T / Z)^T @ Wu slice  -> partial [4096, 512] fp32
Host sums the 4 per-batch fp32 partials and adds b_u.

Schedule notes (from trace iteration): exp throughput is the global
wall -- ScalarE 1 elem/cyc/lane @1.2GHz, DVE 1x from fp32 PSUM @0.96GHz
-- so both engines run near-saturated and everything else fits around
them.  Input DMA goes on ONE queue in strict first-use order (the HBM
fabric is shared; a second queue slows the critical condT transfers).
The kT/qT/fo projections borrow the pv-pool PSUM slot, never a stage
slot (stage slots gate the scores; the stage double-buffer also sets a
latency floor: scores(n+2) waits for exp(n), hence LAG=4 and one
full-tile exp per chunk).  Pushing the PE past ~85% sustained triggers
a chip power-state downclock to 2.0GHz (-17% on every engine), which is
why Z is split rather than all-PE.
"""

import numpy as np
import ml_dtypes

B, T, C = 2, 4096, 512
H, DH = 8, 64
COND = 512
SCALE = (DH // H) ** -0.5  # faithful to reference: 8**-0.5
NCORES = 8
DV = 2 * DH          # per-core head-pair width = 128
CK = COND // 128     # 4 contraction chunks
TJ = T // 128        # 32 key chunks
IB = 512             # query block
NIB = T // IB        # 8
LAG = 4              # chunks the PV/Z consumption trails the scores/exp
EXP_BIAS = -8.0      # constant shift inside exp; cancels in the softmax ratio

# Schraudolph bf16 exp: bits_i16 = round(S*SCH_A + SCH_B) ~ bf16(exp(S-8))
SCH_A = 128.0 / float(np.log(2.0))              # 184.664...
SCH_B = EXP_BIAS * SCH_A + 127.0 * 128.0 - 128.0 * 0.043677

_BUILT = None


def _build_nc():
    import concourse.bass as bass  # noqa: F401
    import concourse.tile as tile
    from concourse import bacc, mybir

    f32 = mybir.dt.float32
    bf16 = mybir.dt.bfloat16
    i16 = mybir.dt.int16

    nc = bacc.Bacc(None)
    condT_d = nc.declare_dram_parameter("condT", [COND, T], bf16, isOutput=False)
    xT_d = nc.declare_dram_parameter("xT", [C, T], bf16, isOutput=False)
    Wq_d = nc.declare_dram_parameter("Wq", [COND, DV], bf16, isOutput=False)
    Wk_d = nc.declare_dram_parameter("Wk", [COND, DV], bf16, isOutput=False)
    Wv_d = nc.declare_dram_parameter("Wv", [C, DV], bf16, isOutput=False)
    Wu_d = nc.declare_dram_parameter("Wu", [DV, C], bf16, isOutput=False)
    out_d = nc.declare_dram_parameter("out", [T, C], f32, isOutput=True)

    Exp = mybir.ActivationFunctionType.Exp
    MULT = mybir.AluOpType.mult
    ADD = mybir.AluOpType.add

    with tile.TileContext(nc) as tc:
        with (
            tc.tile_pool(name="persist", bufs=1) as persist,
            tc.tile_pool(name="work", bufs=3) as work,
            tc.tile_pool(name="outsb", bufs=3) as outsb,
            tc.tile_pool(name="pt_pool", bufs=LAG + 3) as pt_pool,
            tc.tile_pool(name="zaccD_pool", bufs=2) as zaccD_pool,
            tc.tile_pool(name="stage_ps", bufs=2, space="PSUM") as stage_ps,
            tc.tile_pool(name="pv_ps", bufs=2, space="PSUM") as pv_ps,
            tc.tile_pool(name="zb_ps", bufs=2, space="PSUM") as zb_ps,
        ):
            # ------- load inputs (kT-projection dependencies first) -------
            # single sync queue, strict deadline order: the HBM fabric is
            # shared, so a second queue only slows the critical transfers
            Wk_sb = persist.tile([128, CK, DV], bf16)
            nc.sync.dma_start(Wk_sb, Wk_d.rearrange("(co ci) d -> ci co d", ci=128))
            # Wv early: v_proj(0..) runs in the first chunks and its MMs
            # sit ahead of the scores in the PE queue
            Wv_sb = persist.tile([128, CK, DV], bf16)
            nc.sync.dma_start(Wv_sb, Wv_d.rearrange("(co ci) d -> ci co d", ci=128))
            condT_r = condT_d.rearrange("(co ci) t -> ci co t", ci=128)
            condT_sb = persist.tile([128, CK, T], bf16)
            nc.sync.dma_start(condT_sb[:, :, 0:256], condT_r[:, :, 0:256])
            nc.sync.dma_start(condT_sb[:, :, 256:512], condT_r[:, :, 256:512])
            Wq_sb = persist.tile([128, CK, DV], bf16)
            nc.sync.dma_start(Wq_sb, Wq_d.rearrange("(co ci) d -> ci co d", ci=128))
            xT_r = xT_d.rearrange("(co ci) t -> ci co t", ci=128)
            xT_sb = persist.tile([128, CK, T], bf16)
            nc.sync.dma_start(xT_sb[:, :, 0:512], xT_r[:, :, 0:512])
            Wu_sb = persist.tile([128, C], bf16)
            nc.sync.dma_start(Wu_sb, Wu_d[:, :])
            ones_sb = persist.tile([128, 64], bf16)
            nc.vector.memset(ones_sb, 1.0)
            ebias_sb = persist.tile([128, 1], f32)
            nc.vector.memset(ebias_sb, EXP_BIAS)

            # HAM warm-up: keep the PE busy through the initial DMA wait so
            # the clock gate is at 8/8 (2.4GHz) when the first projections
            # run (cold MMs are ~1.6x slower); results are discarded
            warm_sb = persist.tile([128, 512], bf16)
            nc.vector.memset(warm_sb, 0.0)
            warm_ps = zb_ps.tile([64, 512], f32, tag="zb", name="warm")
            for w in range(25):
                nc.tensor.matmul(
                    warm_ps,
                    lhsT=ones_sb,
                    rhs=warm_sb,
                    start=True,
                    stop=True,
                )

            for ts in range(1, T // 512):
                sl = slice(ts * 512, (ts + 1) * 512)
                if ts <= 3:
                    # per-ck pieces: the kT burst's first accumulate can
                    # start on ck0 instead of waiting for the whole slice
                    for ck in range(CK):
                        nc.sync.dma_start(
                            condT_sb[:, ck, sl], condT_r[:, ck, sl]
                        )
                else:
                    nc.sync.dma_start(condT_sb[:, :, sl], condT_r[:, :, sl])
                nc.sync.dma_start(xT_sb[:, :, sl], xT_r[:, :, sl])

            qT_sb = persist.tile([128, T], bf16)  # partitions 0:64 h0 d, 64:128 h1
            kT_sb = persist.tile([128, T], bf16)
            v_sb = persist.tile([128, TJ, DV], bf16)  # [j_inner, j_outer, dv]

            def qk_proj(ts, W_sb, out_sb, copy_eng):
                # one 512-wide t-slice of the q^T (or k^T) projection;
                # borrows the pv-pool spare slot (stage slots are needed
                # every chunk by the scores, so stealing one stalls the PE)
                sl = slice(ts * 512, (ts + 1) * 512)
                p_ps = pv_ps.tile(
                    [128, 512], f32, tag="pv", name=f"pj_{out_sb.tensor.name}_{ts}"
                )
                for ck in range(CK):
                    nc.tensor.matmul(
                        p_ps,
                        lhsT=W_sb[:, ck, :],
                        rhs=condT_sb[:, ck, sl],
                        start=(ck == 0),
                        stop=(ck == CK - 1),
                    )
                if copy_eng == "scalar":
                    nc.scalar.copy(out_sb[:, sl], p_ps)
                else:
                    nc.vector.tensor_copy(out_sb[:, sl], p_ps)

            # ---------------- flat pipelined attention ----------------
            pvs = {}
            zbs = {}
            pts = {}
            zaccDs = {}
            # Z-chunk ownership: DVE bf16 accumulation on odd tj (16/32,
            # folded into zb at block end), PE ones-matmul on even tj --
            # the PE is the binding engine in steady state (92% busy) and
            # the DVE has slack.  (Measured: 16 DVE chunks beats 12 and 14;
            # moving tj 29/31 back to the PE costs more than the block-seam
            # fold-wait it removes.)
            DVE_Z = frozenset(range(1, TJ, 2))
            DVE_Z_FIRST = 1

            def v_proj_chunk(tj):
                # v[j, dv] for one 128-row j chunk; borrows a zb-pool slot
                v_psum = zb_ps.tile([128, 512], f32, tag="zb", name=f"v_psum_{tj}")
                for ck in range(CK):
                    nc.tensor.matmul(
                        v_psum[:, 0:DV],
                        lhsT=xT_sb[:, ck, tj * 128 : (tj + 1) * 128],
                        rhs=Wv_sb[:, ck, :],
                        start=(ck == 0),
                        stop=(ck == CK - 1),
                    )
                if tj % 8 in (2, 5, 7):
                    # chunks whose exp runs on the DVE -> ScalarE is free
                    nc.scalar.copy(v_sb[:, tj, :], v_psum[:, 0:DV])
                else:
                    nc.vector.tensor_copy(v_sb[:, tj, :], v_psum[:, 0:DV])

            pvns = {}

            def finish_block(ib):
                # fold the DVE/GPSIMD-accumulated P sums into the PSUM Z,
                # then normalize; the final projection is spread out over
                # the following block's chunks via fo_step
                pv = pvs.pop(ib)
                zb = zbs.pop(ib)
                zaccD = zaccDs.pop(ib)
                nc.tensor.matmul(
                    zb[0:64, :],
                    lhsT=ones_sb,
                    rhs=zaccD[:, 0, :],
                    start=False,
                    stop=True,
                    tile_position=(0, 0),
                )
                nc.tensor.matmul(
                    zb[64:128, :],
                    lhsT=ones_sb,
                    rhs=zaccD[:, 1, :],
                    start=False,
                    stop=True,
                    tile_position=(0, 64),
                )
                zr = work.tile([128, IB], f32, tag="zr", name=f"zr_{ib}")
                nc.vector.reciprocal_approx_fast(zr, zb)
                pvn = work.tile([128, IB], bf16, tag="pvn", name=f"pvn_{ib}")
                nc.vector.tensor_mul(pvn, pv, zr)
                pvns[ib] = pvn

            def fo_step(ib, isub, copy_eng="scalar", pool=None):
                # matmul + PSUM->SBUF copy (ScalarE on chunks where the DVE
                # runs the exp) + DMA out
                pvn = pvns[ib]
                if pool is None:
                    fo = pv_ps.tile([128, C], f32, tag="pv", name=f"fo_{ib}_{isub}")
                else:
                    fo = pool.tile([128, C], f32, tag="zb", name=f"fo_{ib}_{isub}")
                nc.tensor.matmul(
                    fo,
                    lhsT=pvn[:, isub * 128 : (isub + 1) * 128],
                    rhs=Wu_sb,
                    start=True,
                    stop=True,
                )
                fo_sb = outsb.tile([128, C], f32, tag="fo", name=f"fs_{ib}_{isub}")
                if copy_eng == "scalar":
                    nc.scalar.copy(fo_sb, fo)
                else:
                    nc.vector.tensor_copy(fo_sb, fo)
                t0 = ib * IB + isub * 128
                nc.sync.dma_start(out_d[t0 : t0 + 128, :], fo_sb)
                if isub == IB // 128 - 1:
                    del pvns[ib]

            def consume(n):
                ib, tj = divmod(n, TJ)
                if tj == 0:
                    # allocate at first write so the pool slots stay free
                    # for the projection bursts during the preceding chunks
                    pvs[ib] = pv_ps.tile(
                        [128, IB], f32, tag="pv", name=f"pv_{ib}"
                    )
                    zbs[ib] = zb_ps.tile(
                        [128, IB], f32, tag="zb", name=f"zb_{ib}"
                    )
                    zaccDs[ib] = zaccD_pool.tile(
                        [128, 2, IB], bf16, tag="zaccD", name=f"zaccD_{ib}"
                    )
                pv = pvs[ib]
                zb = zbs[ib]
                pt = pts.pop(n)
                nc.tensor.matmul(
                    pv[0:64, :],
                    lhsT=v_sb[:, tj, 0:64],
                    rhs=pt[:, 0, :],
                    start=(tj == 0),
                    stop=(tj == TJ - 1),
                    tile_position=(0, 0),
                )
                nc.tensor.matmul(
                    pv[64:128, :],
                    lhsT=v_sb[:, tj, 64:128],
                    rhs=pt[:, 1, :],
                    start=(tj == 0),
                    stop=(tj == TJ - 1),
                    tile_position=(0, 64),
                )
                # last block: keep the final Z chunks off the DVE so the
                # epilogue fold/recip never wait on a just-issued accumulate
                if tj not in DVE_Z or (ib == NIB - 1 and tj >= 29):
                    nc.tensor.matmul(
                        zb[0:64, :],
                        lhsT=ones_sb,
                        rhs=pt[:, 0, :],
                        start=(tj == 0),
                        stop=False,
                        tile_position=(0, 0),
                    )
                    nc.tensor.matmul(
                        zb[64:128, :],
                        lhsT=ones_sb,
                        rhs=pt[:, 1, :],
                        start=(tj == 0),
                        stop=False,
                        tile_position=(0, 64),
                    )
                else:
                    zaccD = zaccDs[ib]
                    if tj == DVE_Z_FIRST:
                        nc.vector.tensor_copy(zaccD, pt)
                    else:
                        nc.vector.tensor_add(zaccD, zaccD, pt)
                if tj == TJ - 1:
                    finish_block(ib)

            # consume(m) runs at n = m + LAG
            sched = {}
            for m in range(NIB * TJ):
                sched.setdefault(m + LAG, []).append(m)

            qproj_state = {}

            def q_proj_step(ib, step):
                # one K=128 partial of next block's qT projection; the psum
                # group stays open across several chunks so the PE absorbs
                # it in its per-chunk slack instead of one big bubble
                ts = ib + 1
                if step == 0:
                    qproj_state[ts] = pv_ps.tile(
                        [128, 512], f32, tag="pv", name=f"qp_{ts}"
                    )
                p_ps = qproj_state[ts]
                sl = slice(ts * 512, (ts + 1) * 512)
                nc.tensor.matmul(
                    p_ps,
                    lhsT=Wq_sb[:, step, :],
                    rhs=condT_sb[:, step, sl],
                    start=(step == 0),
                    stop=(step == CK - 1),
                )
                if step == CK - 1:
                    nc.scalar.copy(qT_sb[:, sl], p_ps)
                    del qproj_state[ts]

            N = NIB * TJ
            for n in range(N):
                ib, tj = divmod(n, TJ)
                if n == 0:
                    qk_proj(0, Wk_sb, kT_sb, "scalar")
                    qk_proj(0, Wq_sb, qT_sb, "vector")
                if n < 27 and n % 4 == 2:
                    # kT slice ts=1..7, paced to the condT DMA arrivals so
                    # the burst's LDW wait doesn't head-of-line-block scores
                    ts = (n - 2) // 4 + 1
                    qk_proj(ts, Wk_sb, kT_sb,
                            "scalar" if ts % 2 == 0 else "vector")
                if ib == 0 and tj == 29:
                    # block 0: single burst AFTER the kT bursts release the
                    # pv-pool spare slot (the spread version deadlocked the
                    # slot against the ts=7 kT burst for ~5us)
                    qk_proj(1, Wq_sb, qT_sb, "scalar")
                elif 0 < ib < NIB - 1 and tj in (23, 25, 27, 29):
                    q_proj_step(ib, (tj - 23) // 2)
                if ib > 0 and tj in (5, 7, 13, 15):
                    fo_step(ib - 1, (5, 7, 13, 15).index(tj))
                if n < TJ:
                    v_proj_chunk(n)
                isl = slice(ib * IB, (ib + 1) * IB)
                jsl = slice(tj * 128, (tj + 1) * 128)
                st = stage_ps.tile([128, 2, 512], f32, tag="stage", name=f"st_{n}")
                # scores S^T[j, i] per head; K=64 row-packed (h0 rows 0-63,
                # h1 rows 64-127) -> concurrent on the PE
                nc.tensor.matmul(
                    st[:, 0, :],
                    lhsT=kT_sb[0:64, jsl],
                    rhs=qT_sb[0:64, isl],
                    start=True,
                    stop=True,
                )
                nc.tensor.matmul(
                    st[:, 1, :],
                    lhsT=kT_sb[64:128, jsl],
                    rhs=qT_sb[64:128, isl],
                    start=True,
                    stop=True,
                )
                pt = pt_pool.tile([128, 2, 512], bf16, tag="pt", name=f"pt_{n}")
                if n % 8 in (2, 5, 7) and n != N - 1:
                    # odd chunks: DVE Schraudolph bit-trick exp, full tile
                    nc.vector.tensor_scalar(
                        pt[:, :, :].bitcast(i16),
                        st[:, :, :],
                        SCH_A,
                        SCH_B,
                        MULT,
                        ADD,
                    )
                else:
                    # even chunks: ScalarE exact exp, full tile
                    nc.scalar.activation(pt, st, Exp, bias=ebias_sb[:, :], scale=1.0)
                pts[n] = pt
                for m in sched.get(n, []):
                    consume(m)
            for n in range(N, N + LAG + 1):
                for m in sched.get(n, []):
                    consume(m)
            for isub in range(IB // 128):
                # epilogue: zb banks are free, so run the four final
                # projections through both pools and both copy engines
                fo_step(NIB - 1, isub,
                        "scalar" if isub % 2 == 0 else "vector",
                        pool=zb_ps if isub >= 2 else None)

    nc.compile()
    return nc


def _get_nc():
    global _BUILT
    if _BUILT is None:
        _BUILT = _build_nc()
    return _BUILT


def kernel(x, condition, W_qk, W_v, W_u, b_u):
    from concourse.bass_utils import run_bass_kernel_spmd

    bf = ml_dtypes.bfloat16
    x = np.asarray(x, dtype=np.float32)
    condition = np.asarray(condition, dtype=np.float32)
    W_qk = np.asarray(W_qk, dtype=np.float32)
    W_v = np.asarray(W_v, dtype=np.float32)
    W_u = np.asarray(W_u, dtype=np.float32)
    b_u = np.asarray(b_u, dtype=np.float32)

    Wq = (W_qk[:, : H * DH] * SCALE).astype(bf)
    Wk = W_qk[:, H * DH :].astype(bf)
    Wv = W_v.astype(bf)
    Wu = W_u.astype(bf)
    condT = np.ascontiguousarray(condition.transpose(0, 2, 1)).astype(bf)
    xT = np.ascontiguousarray(x.transpose(0, 2, 1)).astype(bf)

    in_maps = []
    for core in range(NCORES):
        b = core // 4
        hp = core % 4
        ds = slice(hp * DV, (hp + 1) * DV)
        in_maps.append(
            {
                "condT": condT[b],
                "xT": xT[b],
                "Wq": np.ascontiguousarray(Wq[:, ds]),
                "Wk": np.ascontiguousarray(Wk[:, ds]),
                "Wv": np.ascontiguousarray(Wv[:, ds]),
                "Wu": np.ascontiguousarray(Wu[ds, :]),
            }
        )

    nc = _get_nc()
    res = run_bass_kernel_spmd(nc, in_maps, core_ids=list(range(NCORES)))
    results = res.results

    out = np.zeros((B, T, C), dtype=np.float32)
    for core in range(NCORES):
        out[core // 4] += np.asarray(results[core]["out"], dtype=np.float32)
    out += b_u
    return out


# revision 86
# speedup vs baseline: 1.0028x; 1.0028x over previous
"""CrossAttentionBlock kernel for 8 Trainium2 NeuronCores.

Sharding: 16 (batch, head) pairs -> 8 cores, each core owns one batch b and
two heads (2*hp, 2*hp+1).  Per core:
  qT/kT = (Wq/Wk slice)^T-projection of condition[b]   [128=2*64 d, 4096 t]
  v     = x[b] @ Wv slice                               [4096 j, 128 dv]
  S^T   = kT^T-slices @ qT  (per head, row-packed on the PE)
  P     = exp(S^T - 8), one full [128,1024] tile per chunk, alternating
          engines (FD=1024 per op amortizes the per-instruction
          overhead): ScalarE exact exp on 20/32 chunks, DVE Schraudolph
          (bf16 bits = round(S*128*log2e + B)) on 12/32 (n%8 in 2,5,7)
  out^T = v^T @ P^T  accumulated over j (col-packed 2 heads)
  Z     = ones^T @ P^T accumulated over j: PE ones-matmul pair on 20/32
          chunks, DVE bf16 accumulator on 12/32 (folded in at block end)
  final = (out^T / Z)^T @ Wu slice  -> partial [4096, 512] fp32
Host sums the 4 per-batch fp32 partials and adds b_u.

Schedule notes (from trace iteration): exp throughput is the global
wall -- ScalarE 1 elem/cyc/lane @1.2GHz, DVE 1x from fp32 PSUM @0.96GHz
-- so both engines run near-saturated and everything else fits around
them.  Input DMA goes on ONE queue in strict first-use order (the HBM
fabric is shared; a second queue slows the critical condT transfers).
The kT/qT/fo projections borrow the pv-pool PSUM slot, never a stage
slot (stage slots gate the scores; the stage double-buffer also sets a
latency floor: scores(n+2) waits for exp(n), hence LAG=4 and one
full-tile exp per chunk).  Pushing the PE past ~85% sustained triggers
a chip power-state downclock to 2.0GHz (-17% on every engine), which is
why Z is split rather than all-PE.
"""

import numpy as np
import ml_dtypes

B, T, C = 2, 4096, 512
H, DH = 8, 64
COND = 512
SCALE = (DH // H) ** -0.5  # faithful to reference: 8**-0.5
NCORES = 8
DV = 2 * DH          # per-core head-pair width = 128
CK = COND // 128     # 4 contraction chunks
TJ = T // 128        # 32 key chunks
IB = 512             # query block
NIB = T // IB        # 8
LAG = 4              # chunks the PV/Z consumption trails the scores/exp
EXP_BIAS = -8.0      # constant shift inside exp; cancels in the softmax ratio

# Schraudolph bf16 exp: bits_i16 = round(S*SCH_A + SCH_B) ~ bf16(exp(S-8))
SCH_A = 128.0 / float(np.log(2.0))              # 184.664...
SCH_B = EXP_BIAS * SCH_A + 127.0 * 128.0 - 128.0 * 0.043677

_BUILT = None


def _build_nc():
    import concourse.bass as bass  # noqa: F401
    import concourse.tile as tile
    from concourse import bacc, mybir

    f32 = mybir.dt.float32
    bf16 = mybir.dt.bfloat16
    i16 = mybir.dt.int16

    nc = bacc.Bacc(None)
    condT_d = nc.declare_dram_parameter("condT", [COND, T], bf16, isOutput=False)
    xT_d = nc.declare_dram_parameter("xT", [C, T], bf16, isOutput=False)
    Wq_d = nc.declare_dram_parameter("Wq", [COND, DV], bf16, isOutput=False)
    Wk_d = nc.declare_dram_parameter("Wk", [COND, DV], bf16, isOutput=False)
    Wv_d = nc.declare_dram_parameter("Wv", [C, DV], bf16, isOutput=False)
    Wu_d = nc.declare_dram_parameter("Wu", [DV, C], bf16, isOutput=False)
    out_d = nc.declare_dram_parameter("out", [T, C], f32, isOutput=True)

    Exp = mybir.ActivationFunctionType.Exp
    MULT = mybir.AluOpType.mult
    ADD = mybir.AluOpType.add

    with tile.TileContext(nc) as tc:
        with (
            tc.tile_pool(name="persist", bufs=1) as persist,
            tc.tile_pool(name="work", bufs=3) as work,
            tc.tile_pool(name="outsb", bufs=3) as outsb,
            tc.tile_pool(name="pt_pool", bufs=LAG + 3) as pt_pool,
            tc.tile_pool(name="zaccD_pool", bufs=2) as zaccD_pool,
            tc.tile_pool(name="stage_ps", bufs=2, space="PSUM") as stage_ps,
            tc.tile_pool(name="pv_ps", bufs=2, space="PSUM") as pv_ps,
            tc.tile_pool(name="zb_ps", bufs=2, space="PSUM") as zb_ps,
        ):
            # ------- load inputs (kT-projection dependencies first) -------
            # single sync queue, strict deadline order: the HBM fabric is
            # shared, so a second queue only slows the critical transfers
            Wk_sb = persist.tile([128, CK, DV], bf16)
            nc.sync.dma_start(Wk_sb, Wk_d.rearrange("(co ci) d -> ci co d", ci=128))
            # Wv early: v_proj(0..) runs in the first chunks and its MMs
            # sit ahead of the scores in the PE queue
            Wv_sb = persist.tile([128, CK, DV], bf16)
            nc.sync.dma_start(Wv_sb, Wv_d.rearrange("(co ci) d -> ci co d", ci=128))
            condT_r = condT_d.rearrange("(co ci) t -> ci co t", ci=128)
            condT_sb = persist.tile([128, CK, T], bf16)
            # per-ck pieces: the kT0 projection's first accumulate starts
            # on ck0 (~128KB) instead of waiting for the full 512KB slice
            for ck in range(CK):
                nc.sync.dma_start(condT_sb[:, ck, 0:512], condT_r[:, ck, 0:512])
            Wq_sb = persist.tile([128, CK, DV], bf16)
            nc.sync.dma_start(Wq_sb, Wq_d.rearrange("(co ci) d -> ci co d", ci=128))
            xT_r = xT_d.rearrange("(co ci) t -> ci co t", ci=128)
            xT_sb = persist.tile([128, CK, T], bf16)
            nc.sync.dma_start(xT_sb[:, :, 0:512], xT_r[:, :, 0:512])
            Wu_sb = persist.tile([128, C], bf16)
            nc.sync.dma_start(Wu_sb, Wu_d[:, :])
            ones_sb = persist.tile([128, 64], bf16)
            nc.vector.memset(ones_sb, 1.0)
            ebias_sb = persist.tile([128, 1], f32)
            nc.vector.memset(ebias_sb, EXP_BIAS)

            for ts in range(1, T // 512):
                sl = slice(ts * 512, (ts + 1) * 512)
                # per-ck pieces: each kT burst's first accumulate can
                # start on ck0 instead of waiting for the whole slice
                for ck in range(CK):
                    nc.sync.dma_start(
                        condT_sb[:, ck, sl], condT_r[:, ck, sl]
                    )
                nc.sync.dma_start(xT_sb[:, :, sl], xT_r[:, :, sl])

            qT_sb = persist.tile([128, T], bf16)  # partitions 0:64 h0 d, 64:128 h1
            kT_sb = persist.tile([128, T], bf16)
            v_sb = persist.tile([128, TJ, DV], bf16)  # [j_inner, j_outer, dv]

            def qk_proj(ts, W_sb, out_sb, copy_eng):
                # one 512-wide t-slice of the q^T (or k^T) projection;
                # borrows the pv-pool spare slot (stage slots are needed
                # every chunk by the scores, so stealing one stalls the PE)
                sl = slice(ts * 512, (ts + 1) * 512)
                p_ps = pv_ps.tile(
                    [128, 512], f32, tag="pv", name=f"pj_{out_sb.tensor.name}_{ts}"
                )
                for ck in range(CK):
                    nc.tensor.matmul(
                        p_ps,
                        lhsT=W_sb[:, ck, :],
                        rhs=condT_sb[:, ck, sl],
                        start=(ck == 0),
                        stop=(ck == CK - 1),
                    )
                if copy_eng == "scalar":
                    nc.scalar.copy(out_sb[:, sl], p_ps)
                else:
                    nc.vector.tensor_copy(out_sb[:, sl], p_ps)

            # ---------------- flat pipelined attention ----------------
            pvs = {}
            zbs = {}
            pts = {}
            zaccDs = {}
            # Z-chunk ownership: DVE bf16 accumulation on odd tj (16/32,
            # folded into zb at block end), PE ones-matmul on even tj --
            # the PE is the binding engine in steady state (92% busy) and
            # the DVE has slack.  (Measured: 16 DVE chunks beats 12 and 14;
            # moving tj 29/31 back to the PE costs more than the block-seam
            # fold-wait it removes.)
            DVE_Z = frozenset(range(1, TJ, 2))
            DVE_Z_FIRST = 1

            def v_proj_chunk(tj):
                # v[j, dv] for one 128-row j chunk; borrows a zb-pool slot
                v_psum = zb_ps.tile([128, 512], f32, tag="zb", name=f"v_psum_{tj}")
                for ck in range(CK):
                    nc.tensor.matmul(
                        v_psum[:, 0:DV],
                        lhsT=xT_sb[:, ck, tj * 128 : (tj + 1) * 128],
                        rhs=Wv_sb[:, ck, :],
                        start=(ck == 0),
                        stop=(ck == CK - 1),
                    )
                if tj % 8 in (2, 5, 7):
                    # chunks whose exp runs on the DVE -> ScalarE is free
                    nc.scalar.copy(v_sb[:, tj, :], v_psum[:, 0:DV])
                else:
                    nc.vector.tensor_copy(v_sb[:, tj, :], v_psum[:, 0:DV])

            pvns = {}

            def finish_block(ib):
                # fold the DVE/GPSIMD-accumulated P sums into the PSUM Z,
                # then normalize; the final projection is spread out over
                # the following block's chunks via fo_step
                pv = pvs.pop(ib)
                zb = zbs.pop(ib)
                zaccD = zaccDs.pop(ib)
                nc.tensor.matmul(
                    zb[0:64, :],
                    lhsT=ones_sb,
                    rhs=zaccD[:, 0, :],
                    start=False,
                    stop=True,
                    tile_position=(0, 0),
                )
                nc.tensor.matmul(
                    zb[64:128, :],
                    lhsT=ones_sb,
                    rhs=zaccD[:, 1, :],
                    start=False,
                    stop=True,
                    tile_position=(0, 64),
                )
                zr = work.tile([128, IB], f32, tag="zr", name=f"zr_{ib}")
                nc.vector.reciprocal_approx_fast(zr, zb)
                pvn = work.tile([128, IB], bf16, tag="pvn", name=f"pvn_{ib}")
                nc.vector.tensor_mul(pvn, pv, zr)
                pvns[ib] = pvn

            def fo_step(ib, isub, copy_eng="scalar", pool=None):
                # matmul + PSUM->SBUF copy (ScalarE on chunks where the DVE
                # runs the exp) + DMA out
                pvn = pvns[ib]
                if pool is None:
                    fo = pv_ps.tile([128, C], f32, tag="pv", name=f"fo_{ib}_{isub}")
                else:
                    fo = pool.tile([128, C], f32, tag="zb", name=f"fo_{ib}_{isub}")
                nc.tensor.matmul(
                    fo,
                    lhsT=pvn[:, isub * 128 : (isub + 1) * 128],
                    rhs=Wu_sb,
                    start=True,
                    stop=True,
                )
                fo_sb = outsb.tile([128, C], f32, tag="fo", name=f"fs_{ib}_{isub}")
                if copy_eng == "scalar":
                    nc.scalar.copy(fo_sb, fo)
                else:
                    nc.vector.tensor_copy(fo_sb, fo)
                t0 = ib * IB + isub * 128
                nc.sync.dma_start(out_d[t0 : t0 + 128, :], fo_sb)
                if isub == IB // 128 - 1:
                    del pvns[ib]

            def consume(n):
                ib, tj = divmod(n, TJ)
                if tj == 0:
                    # allocate at first write so the pool slots stay free
                    # for the projection bursts during the preceding chunks
                    pvs[ib] = pv_ps.tile(
                        [128, IB], f32, tag="pv", name=f"pv_{ib}"
                    )
                    zbs[ib] = zb_ps.tile(
                        [128, IB], f32, tag="zb", name=f"zb_{ib}"
                    )
                    zaccDs[ib] = zaccD_pool.tile(
                        [128, 2, IB], bf16, tag="zaccD", name=f"zaccD_{ib}"
                    )
                pv = pvs[ib]
                zb = zbs[ib]
                pt = pts.pop(n)
                nc.tensor.matmul(
                    pv[0:64, :],
                    lhsT=v_sb[:, tj, 0:64],
                    rhs=pt[:, 0, :],
                    start=(tj == 0),
                    stop=(tj == TJ - 1),
                    tile_position=(0, 0),
                )
                nc.tensor.matmul(
                    pv[64:128, :],
                    lhsT=v_sb[:, tj, 64:128],
                    rhs=pt[:, 1, :],
                    start=(tj == 0),
                    stop=(tj == TJ - 1),
                    tile_position=(0, 64),
                )
                # last block: keep the final Z chunks off the DVE so the
                # epilogue fold/recip never wait on a just-issued accumulate
                if tj not in DVE_Z or (ib == NIB - 1 and tj >= 29):
                    nc.tensor.matmul(
                        zb[0:64, :],
                        lhsT=ones_sb,
                        rhs=pt[:, 0, :],
                        start=(tj == 0),
                        stop=False,
                        tile_position=(0, 0),
                    )
                    nc.tensor.matmul(
                        zb[64:128, :],
                        lhsT=ones_sb,
                        rhs=pt[:, 1, :],
                        start=(tj == 0),
                        stop=False,
                        tile_position=(0, 64),
                    )
                else:
                    zaccD = zaccDs[ib]
                    if tj == DVE_Z_FIRST:
                        nc.vector.tensor_copy(zaccD, pt)
                    else:
                        nc.vector.tensor_add(zaccD, zaccD, pt)
                if tj == TJ - 1:
                    finish_block(ib)

            # consume(m) runs at n = m + LAG
            sched = {}
            for m in range(NIB * TJ):
                sched.setdefault(m + LAG, []).append(m)

            qproj_state = {}

            def q_proj_step(ib, step):
                # one K=128 partial of next block's qT projection; the psum
                # group stays open across several chunks so the PE absorbs
                # it in its per-chunk slack instead of one big bubble
                ts = ib + 1
                if step == 0:
                    qproj_state[ts] = pv_ps.tile(
                        [128, 512], f32, tag="pv", name=f"qp_{ts}"
                    )
                p_ps = qproj_state[ts]
                sl = slice(ts * 512, (ts + 1) * 512)
                nc.tensor.matmul(
                    p_ps,
                    lhsT=Wq_sb[:, step, :],
                    rhs=condT_sb[:, step, sl],
                    start=(step == 0),
                    stop=(step == CK - 1),
                )
                if step == CK - 1:
                    nc.scalar.copy(qT_sb[:, sl], p_ps)
                    del qproj_state[ts]

            N = NIB * TJ
            for n in range(N):
                ib, tj = divmod(n, TJ)
                if n == 0:
                    qk_proj(0, Wk_sb, kT_sb, "scalar")
                    qk_proj(0, Wq_sb, qT_sb, "vector")
                if n < 27 and n % 4 == 2:
                    # kT slice ts=1..7, paced to the condT DMA arrivals so
                    # the burst's LDW wait doesn't head-of-line-block scores
                    ts = (n - 2) // 4 + 1
                    qk_proj(ts, Wk_sb, kT_sb,
                            "scalar" if ts % 2 == 0 else "vector")
                if ib == 0 and tj == 29:
                    # block 0: single burst AFTER the kT bursts release the
                    # pv-pool spare slot (the spread version deadlocked the
                    # slot against the ts=7 kT burst for ~5us)
                    qk_proj(1, Wq_sb, qT_sb, "scalar")
                elif 0 < ib < NIB - 1 and tj in (23, 25, 27, 29):
                    q_proj_step(ib, (tj - 23) // 2)
                if ib > 0 and tj in (5, 7, 13, 15):
                    fo_step(ib - 1, (5, 7, 13, 15).index(tj))
                if n < TJ:
                    v_proj_chunk(n)
                isl = slice(ib * IB, (ib + 1) * IB)
                jsl = slice(tj * 128, (tj + 1) * 128)
                st = stage_ps.tile([128, 2, 512], f32, tag="stage", name=f"st_{n}")
                # scores S^T[j, i] per head; K=64 row-packed (h0 rows 0-63,
                # h1 rows 64-127) -> concurrent on the PE
                nc.tensor.matmul(
                    st[:, 0, :],
                    lhsT=kT_sb[0:64, jsl],
                    rhs=qT_sb[0:64, isl],
                    start=True,
                    stop=True,
                )
                nc.tensor.matmul(
                    st[:, 1, :],
                    lhsT=kT_sb[64:128, jsl],
                    rhs=qT_sb[64:128, isl],
                    start=True,
                    stop=True,
                )
                pt = pt_pool.tile([128, 2, 512], bf16, tag="pt", name=f"pt_{n}")
                if n % 8 in (2, 5, 7) and n != N - 1:
                    # odd chunks: DVE Schraudolph bit-trick exp, full tile
                    nc.vector.tensor_scalar(
                        pt[:, :, :].bitcast(i16),
                        st[:, :, :],
                        SCH_A,
                        SCH_B,
                        MULT,
                        ADD,
                    )
                else:
                    # even chunks: ScalarE exact exp, full tile
                    nc.scalar.activation(pt, st, Exp, bias=ebias_sb[:, :], scale=1.0)
                pts[n] = pt
                for m in sched.get(n, []):
                    consume(m)
            for n in range(N, N + LAG + 1):
                for m in sched.get(n, []):
                    consume(m)
            for isub in range(IB // 128):
                # epilogue: zb banks are free, so run the four final
                # projections through both pools and both copy engines
                fo_step(NIB - 1, isub,
                        "scalar" if isub % 2 == 0 else "vector",
                        pool=zb_ps if isub >= 2 else None)

    nc.compile()
    return nc


def _get_nc():
    global _BUILT
    if _BUILT is None:
        _BUILT = _build_nc()
    return _BUILT


def kernel(x, condition, W_qk, W_v, W_u, b_u):
    from concourse.bass_utils import run_bass_kernel_spmd

    bf = ml_dtypes.bfloat16
    x = np.asarray(x, dtype=np.float32)
    condition = np.asarray(condition, dtype=np.float32)
    W_qk = np.asarray(W_qk, dtype=np.float32)
    W_v = np.asarray(W_v, dtype=np.float32)
    W_u = np.asarray(W_u, dtype=np.float32)
    b_u = np.asarray(b_u, dtype=np.float32)

    Wq = (W_qk[:, : H * DH] * SCALE).astype(bf)
    Wk = W_qk[:, H * DH :].astype(bf)
    Wv = W_v.astype(bf)
    Wu = W_u.astype(bf)
    condT = np.ascontiguousarray(condition.transpose(0, 2, 1)).astype(bf)
    xT = np.ascontiguousarray(x.transpose(0, 2, 1)).astype(bf)

    in_maps = []
    for core in range(NCORES):
        b = core // 4
        hp = core % 4
        ds = slice(hp * DV, (hp + 1) * DV)
        in_maps.append(
            {
                "condT": condT[b],
                "xT": xT[b],
                "Wq": np.ascontiguousarray(Wq[:, ds]),
                "Wk": np.ascontiguousarray(Wk[:, ds]),
                "Wv": np.ascontiguousarray(Wv[:, ds]),
                "Wu": np.ascontiguousarray(Wu[ds, :]),
            }
        )

    nc = _get_nc()
    res = run_bass_kernel_spmd(nc, in_maps, core_ids=list(range(NCORES)))
    results = res.results

    out = np.zeros((B, T, C), dtype=np.float32)
    for core in range(NCORES):
        out[core // 4] += np.asarray(results[core]["out"], dtype=np.float32)
    out += b_u
    return out


# revision 88
# speedup vs baseline: 1.0104x; 1.0076x over previous
"""CrossAttentionBlock kernel for 8 Trainium2 NeuronCores.

Sharding: 16 (batch, head) pairs -> 8 cores, each core owns one batch b and
two heads (2*hp, 2*hp+1).  Per core:
  qT/kT = (Wq/Wk slice)^T-projection of condition[b]   [128=2*64 d, 4096 t]
  v     = x[b] @ Wv slice                               [4096 j, 128 dv]
  S^T   = kT^T-slices @ qT  (per head, row-packed on the PE)
  P     = exp(S^T - 8), one full [128,1024] tile per chunk, alternating
          engines (FD=1024 per op amortizes the per-instruction
          overhead): ScalarE exact exp on 20/32 chunks, DVE Schraudolph
          (bf16 bits = round(S*128*log2e + B)) on 12/32 (n%8 in 2,5,7)
  out^T = v^T @ P^T  accumulated over j (col-packed 2 heads)
  Z     = ones^T @ P^T accumulated over j: PE ones-matmul pair on 20/32
          chunks, DVE bf16 accumulator on 12/32 (folded in at block end)
  final = (out^T / Z)^T @ Wu slice  -> partial [4096, 512] fp32
Host sums the 4 per-batch fp32 partials and adds b_u.

Schedule notes (from trace iteration): exp throughput is the global
wall -- ScalarE 1 elem/cyc/lane @1.2GHz, DVE 1x from fp32 PSUM @0.96GHz
-- so both engines run near-saturated and everything else fits around
them.  Input DMA goes on ONE queue in strict first-use order (the HBM
fabric is shared; a second queue slows the critical condT transfers).
The kT/qT/fo projections borrow the pv-pool PSUM slot, never a stage
slot (stage slots gate the scores; the stage double-buffer also sets a
latency floor: scores(n+2) waits for exp(n), hence LAG=4 and one
full-tile exp per chunk).  Pushing the PE past ~85% sustained triggers
a chip power-state downclock to 2.0GHz (-17% on every engine), which is
why Z is split rather than all-PE.
"""

import numpy as np
import ml_dtypes

B, T, C = 2, 4096, 512
H, DH = 8, 64
COND = 512
SCALE = (DH // H) ** -0.5  # faithful to reference: 8**-0.5
NCORES = 8
DV = 2 * DH          # per-core head-pair width = 128
CK = COND // 128     # 4 contraction chunks
TJ = T // 128        # 32 key chunks
IB = 512             # query block
NIB = T // IB        # 8
LAG = 4              # chunks the PV/Z consumption trails the scores/exp
EXP_BIAS = -8.0      # constant shift inside exp; cancels in the softmax ratio

# Schraudolph bf16 exp: bits_i16 = round(S*SCH_A + SCH_B) ~ bf16(exp(S-8))
SCH_A = 128.0 / float(np.log(2.0))              # 184.664...
SCH_B = EXP_BIAS * SCH_A + 127.0 * 128.0 - 128.0 * 0.043677

_BUILT = None


def _build_nc():
    import concourse.bass as bass  # noqa: F401
    import concourse.tile as tile
    from concourse import bacc, mybir

    f32 = mybir.dt.float32
    bf16 = mybir.dt.bfloat16
    i16 = mybir.dt.int16

    nc = bacc.Bacc(None)
    condT_d = nc.declare_dram_parameter("condT", [COND, T], bf16, isOutput=False)
    xT_d = nc.declare_dram_parameter("xT", [C, T], bf16, isOutput=False)
    Wq_d = nc.declare_dram_parameter("Wq", [COND, DV], bf16, isOutput=False)
    Wk_d = nc.declare_dram_parameter("Wk", [COND, DV], bf16, isOutput=False)
    Wv_d = nc.declare_dram_parameter("Wv", [C, DV], bf16, isOutput=False)
    Wu_d = nc.declare_dram_parameter("Wu", [DV, C], bf16, isOutput=False)
    out_d = nc.declare_dram_parameter("out", [T, C], f32, isOutput=True)

    Exp = mybir.ActivationFunctionType.Exp
    MULT = mybir.AluOpType.mult
    ADD = mybir.AluOpType.add

    with tile.TileContext(nc) as tc:
        with (
            tc.tile_pool(name="persist", bufs=1) as persist,
            tc.tile_pool(name="work", bufs=3) as work,
            tc.tile_pool(name="outsb", bufs=3) as outsb,
            tc.tile_pool(name="pt_pool", bufs=LAG + 3) as pt_pool,
            tc.tile_pool(name="zaccD_pool", bufs=2) as zaccD_pool,
            tc.tile_pool(name="stage_ps", bufs=2, space="PSUM") as stage_ps,
            tc.tile_pool(name="pv_ps", bufs=2, space="PSUM") as pv_ps,
            tc.tile_pool(name="zb_ps", bufs=2, space="PSUM") as zb_ps,
        ):
            # ------- load inputs (kT-projection dependencies first) -------
            # single sync queue, strict deadline order: the HBM fabric is
            # shared, so a second queue only slows the critical transfers
            Wk_sb = persist.tile([128, CK, DV], bf16)
            nc.sync.dma_start(Wk_sb, Wk_d.rearrange("(co ci) d -> ci co d", ci=128))
            # Wv early: v_proj(0..) runs in the first chunks and its MMs
            # sit ahead of the scores in the PE queue
            Wv_sb = persist.tile([128, CK, DV], bf16)
            nc.sync.dma_start(Wv_sb, Wv_d.rearrange("(co ci) d -> ci co d", ci=128))
            condT_r = condT_d.rearrange("(co ci) t -> ci co t", ci=128)
            condT_sb = persist.tile([128, CK, T], bf16)
            nc.sync.dma_start(condT_sb[:, :, 0:256], condT_r[:, :, 0:256])
            nc.sync.dma_start(condT_sb[:, :, 256:512], condT_r[:, :, 256:512])
            Wq_sb = persist.tile([128, CK, DV], bf16)
            nc.sync.dma_start(Wq_sb, Wq_d.rearrange("(co ci) d -> ci co d", ci=128))
            xT_r = xT_d.rearrange("(co ci) t -> ci co t", ci=128)
            xT_sb = persist.tile([128, CK, T], bf16)
            nc.sync.dma_start(xT_sb[:, :, 0:512], xT_r[:, :, 0:512])
            Wu_sb = persist.tile([128, C], bf16)
            nc.sync.dma_start(Wu_sb, Wu_d[:, :])
            ones_sb = persist.tile([128, 64], bf16)
            nc.vector.memset(ones_sb, 1.0)
            ebias_sb = persist.tile([128, 1], f32)
            nc.vector.memset(ebias_sb, EXP_BIAS)

            for ts in range(1, T // 512):
                sl = slice(ts * 512, (ts + 1) * 512)
                if ts <= 3:
                    # per-ck pieces: the kT burst's first accumulate can
                    # start on ck0 instead of waiting for the whole slice
                    for ck in range(CK):
                        nc.sync.dma_start(
                            condT_sb[:, ck, sl], condT_r[:, ck, sl]
                        )
                else:
                    nc.sync.dma_start(condT_sb[:, :, sl], condT_r[:, :, sl])
                nc.sync.dma_start(xT_sb[:, :, sl], xT_r[:, :, sl])

            qT_sb = persist.tile([128, T], bf16)  # partitions 0:64 h0 d, 64:128 h1
            kT_sb = persist.tile([128, T], bf16)
            v_sb = persist.tile([128, TJ, DV], bf16)  # [j_inner, j_outer, dv]

            def qk_proj(ts, W_sb, out_sb, copy_eng):
                # one 512-wide t-slice of the q^T (or k^T) projection;
                # borrows the pv-pool spare slot (stage slots are needed
                # every chunk by the scores, so stealing one stalls the PE)
                sl = slice(ts * 512, (ts + 1) * 512)
                p_ps = pv_ps.tile(
                    [128, 512], f32, tag="pv", name=f"pj_{out_sb.tensor.name}_{ts}"
                )
                for ck in range(CK):
                    nc.tensor.matmul(
                        p_ps,
                        lhsT=W_sb[:, ck, :],
                        rhs=condT_sb[:, ck, sl],
                        start=(ck == 0),
                        stop=(ck == CK - 1),
                    )
                if copy_eng == "scalar":
                    nc.scalar.copy(out_sb[:, sl], p_ps)
                else:
                    nc.vector.tensor_copy(out_sb[:, sl], p_ps)

            # ---------------- flat pipelined attention ----------------
            pvs = {}
            zbs = {}
            pts = {}
            zaccDs = {}
            # Z-chunk ownership: DVE bf16 accumulation on odd tj (16/32,
            # folded into zb at block end), PE ones-matmul on even tj --
            # the PE is the binding engine in steady state (92% busy) and
            # the DVE has slack.  (Measured: 16 DVE chunks beats 12 and 14;
            # moving tj 29/31 back to the PE costs more than the block-seam
            # fold-wait it removes.)
            DVE_Z = frozenset(range(1, TJ, 2))
            DVE_Z_FIRST = 1

            def v_proj_chunk(tj):
                # v[j, dv] for one 128-row j chunk; borrows a zb-pool slot
                v_psum = zb_ps.tile([128, 512], f32, tag="zb", name=f"v_psum_{tj}")
                for ck in range(CK):
                    nc.tensor.matmul(
                        v_psum[:, 0:DV],
                        lhsT=xT_sb[:, ck, tj * 128 : (tj + 1) * 128],
                        rhs=Wv_sb[:, ck, :],
                        start=(ck == 0),
                        stop=(ck == CK - 1),
                    )
                if tj % 8 in (2, 5, 7):
                    # chunks whose exp runs on the DVE -> ScalarE is free
                    nc.scalar.copy(v_sb[:, tj, :], v_psum[:, 0:DV])
                else:
                    nc.vector.tensor_copy(v_sb[:, tj, :], v_psum[:, 0:DV])

            pvns = {}

            def finish_block(ib):
                # fold the DVE/GPSIMD-accumulated P sums into the PSUM Z,
                # then normalize; the final projection is spread out over
                # the following block's chunks via fo_step
                pv = pvs.pop(ib)
                zb = zbs.pop(ib)
                zaccD = zaccDs.pop(ib)
                nc.tensor.matmul(
                    zb[0:64, :],
                    lhsT=ones_sb,
                    rhs=zaccD[:, 0, :],
                    start=False,
                    stop=True,
                    tile_position=(0, 0),
                )
                nc.tensor.matmul(
                    zb[64:128, :],
                    lhsT=ones_sb,
                    rhs=zaccD[:, 1, :],
                    start=False,
                    stop=True,
                    tile_position=(0, 64),
                )
                zr = work.tile([128, IB], f32, tag="zr", name=f"zr_{ib}")
                nc.vector.reciprocal_approx_fast(zr, zb)
                pvn = work.tile([128, IB], bf16, tag="pvn", name=f"pvn_{ib}")
                nc.vector.tensor_mul(pvn, pv, zr)
                pvns[ib] = pvn

            def fo_step(ib, isub, copy_eng="scalar", pool=None):
                # matmul + PSUM->SBUF copy (ScalarE on chunks where the DVE
                # runs the exp) + DMA out
                pvn = pvns[ib]
                if pool is None:
                    fo = pv_ps.tile([128, C], f32, tag="pv", name=f"fo_{ib}_{isub}")
                else:
                    fo = pool.tile([128, C], f32, tag="zb", name=f"fo_{ib}_{isub}")
                nc.tensor.matmul(
                    fo,
                    lhsT=pvn[:, isub * 128 : (isub + 1) * 128],
                    rhs=Wu_sb,
                    start=True,
                    stop=True,
                )
                fo_sb = outsb.tile([128, C], f32, tag="fo", name=f"fs_{ib}_{isub}")
                if copy_eng == "scalar":
                    nc.scalar.copy(fo_sb, fo)
                else:
                    nc.vector.tensor_copy(fo_sb, fo)
                t0 = ib * IB + isub * 128
                nc.sync.dma_start(out_d[t0 : t0 + 128, :], fo_sb)
                if isub == IB // 128 - 1:
                    del pvns[ib]

            def consume(n):
                ib, tj = divmod(n, TJ)
                if tj == 0:
                    # allocate at first write so the pool slots stay free
                    # for the projection bursts during the preceding chunks
                    pvs[ib] = pv_ps.tile(
                        [128, IB], f32, tag="pv", name=f"pv_{ib}"
                    )
                    zbs[ib] = zb_ps.tile(
                        [128, IB], f32, tag="zb", name=f"zb_{ib}"
                    )
                    zaccDs[ib] = zaccD_pool.tile(
                        [128, 2, IB], bf16, tag="zaccD", name=f"zaccD_{ib}"
                    )
                pv = pvs[ib]
                zb = zbs[ib]
                pt = pts.pop(n)
                nc.tensor.matmul(
                    pv[0:64, :],
                    lhsT=v_sb[:, tj, 0:64],
                    rhs=pt[:, 0, :],
                    start=(tj == 0),
                    stop=(tj == TJ - 1),
                    tile_position=(0, 0),
                )
                nc.tensor.matmul(
                    pv[64:128, :],
                    lhsT=v_sb[:, tj, 64:128],
                    rhs=pt[:, 1, :],
                    start=(tj == 0),
                    stop=(tj == TJ - 1),
                    tile_position=(0, 64),
                )
                # last block: keep the final Z chunks off the DVE so the
                # epilogue fold/recip never wait on a just-issued accumulate
                if tj not in DVE_Z or (ib == NIB - 1 and tj >= 29):
                    nc.tensor.matmul(
                        zb[0:64, :],
                        lhsT=ones_sb,
                        rhs=pt[:, 0, :],
                        start=(tj == 0),
                        stop=False,
                        tile_position=(0, 0),
                    )
                    nc.tensor.matmul(
                        zb[64:128, :],
                        lhsT=ones_sb,
                        rhs=pt[:, 1, :],
                        start=(tj == 0),
                        stop=False,
                        tile_position=(0, 64),
                    )
                else:
                    zaccD = zaccDs[ib]
                    if tj == DVE_Z_FIRST:
                        nc.vector.tensor_copy(zaccD, pt)
                    else:
                        nc.vector.tensor_add(zaccD, zaccD, pt)
                if tj == TJ - 1:
                    finish_block(ib)

            # consume(m) runs at n = m + LAG
            sched = {}
            for m in range(NIB * TJ):
                sched.setdefault(m + LAG, []).append(m)

            qproj_state = {}

            def q_proj_step(ib, step):
                # one K=128 partial of next block's qT projection; the psum
                # group stays open across several chunks so the PE absorbs
                # it in its per-chunk slack instead of one big bubble
                ts = ib + 1
                if step == 0:
                    qproj_state[ts] = pv_ps.tile(
                        [128, 512], f32, tag="pv", name=f"qp_{ts}"
                    )
                p_ps = qproj_state[ts]
                sl = slice(ts * 512, (ts + 1) * 512)
                nc.tensor.matmul(
                    p_ps,
                    lhsT=Wq_sb[:, step, :],
                    rhs=condT_sb[:, step, sl],
                    start=(step == 0),
                    stop=(step == CK - 1),
                )
                if step == CK - 1:
                    nc.scalar.copy(qT_sb[:, sl], p_ps)
                    del qproj_state[ts]

            N = NIB * TJ
            for n in range(N):
                ib, tj = divmod(n, TJ)
                if n == 0:
                    qk_proj(0, Wk_sb, kT_sb, "scalar")
                    qk_proj(0, Wq_sb, qT_sb, "vector")
                if n < 27 and n % 4 == 2:
                    # kT slice ts=1..7, paced to the condT DMA arrivals so
                    # the burst's LDW wait doesn't head-of-line-block scores
                    ts = (n - 2) // 4 + 1
                    qk_proj(ts, Wk_sb, kT_sb,
                            "scalar" if ts % 2 == 0 else "vector")
                if ib == 0 and tj == 29:
                    # block 0: single burst AFTER the kT bursts release the
                    # pv-pool spare slot (the spread version deadlocked the
                    # slot against the ts=7 kT burst for ~5us)
                    qk_proj(1, Wq_sb, qT_sb, "scalar")
                elif 0 < ib < NIB - 1 and tj in (23, 25, 27, 29):
                    q_proj_step(ib, (tj - 23) // 2)
                if ib > 0 and tj in (5, 7, 13, 15):
                    fo_step(ib - 1, (5, 7, 13, 15).index(tj))
                if n < TJ:
                    v_proj_chunk(n)
                isl = slice(ib * IB, (ib + 1) * IB)
                jsl = slice(tj * 128, (tj + 1) * 128)
                st = stage_ps.tile([128, 2, 512], f32, tag="stage", name=f"st_{n}")
                # scores S^T[j, i] per head; K=64 row-packed (h0 rows 0-63,
                # h1 rows 64-127) -> concurrent on the PE
                nc.tensor.matmul(
                    st[:, 0, :],
                    lhsT=kT_sb[0:64, jsl],
                    rhs=qT_sb[0:64, isl],
                    start=True,
                    stop=True,
                )
                nc.tensor.matmul(
                    st[:, 1, :],
                    lhsT=kT_sb[64:128, jsl],
                    rhs=qT_sb[64:128, isl],
                    start=True,
                    stop=True,
                )
                pt = pt_pool.tile([128, 2, 512], bf16, tag="pt", name=f"pt_{n}")
                if n % 8 in (2, 5, 7) and n != N - 1:
                    # odd chunks: DVE Schraudolph bit-trick exp, full tile
                    nc.vector.tensor_scalar(
                        pt[:, :, :].bitcast(i16),
                        st[:, :, :],
                        SCH_A,
                        SCH_B,
                        MULT,
                        ADD,
                    )
                else:
                    # even chunks: ScalarE exact exp, full tile
                    nc.scalar.activation(pt, st, Exp, bias=ebias_sb[:, :], scale=1.0)
                pts[n] = pt
                for m in sched.get(n, []):
                    consume(m)
            for n in range(N, N + LAG + 1):
                for m in sched.get(n, []):
                    consume(m)
            for isub in range(IB // 128):
                # epilogue: zb banks are free, so run the four final
                # projections through both pools and both copy engines
                fo_step(NIB - 1, isub,
                        "scalar" if isub % 2 == 0 else "vector",
                        pool=zb_ps if isub >= 2 else None)

    nc.compile()
    return nc


def _get_nc():
    global _BUILT
    if _BUILT is None:
        _BUILT = _build_nc()
    return _BUILT


def kernel(x, condition, W_qk, W_v, W_u, b_u):
    from concourse.bass_utils import run_bass_kernel_spmd

    bf = ml_dtypes.bfloat16
    x = np.asarray(x, dtype=np.float32)
    condition = np.asarray(condition, dtype=np.float32)
    W_qk = np.asarray(W_qk, dtype=np.float32)
    W_v = np.asarray(W_v, dtype=np.float32)
    W_u = np.asarray(W_u, dtype=np.float32)
    b_u = np.asarray(b_u, dtype=np.float32)

    Wq = (W_qk[:, : H * DH] * SCALE).astype(bf)
    Wk = W_qk[:, H * DH :].astype(bf)
    Wv = W_v.astype(bf)
    Wu = W_u.astype(bf)
    condT = np.ascontiguousarray(condition.transpose(0, 2, 1)).astype(bf)
    xT = np.ascontiguousarray(x.transpose(0, 2, 1)).astype(bf)

    in_maps = []
    for core in range(NCORES):
        b = core // 4
        hp = core % 4
        ds = slice(hp * DV, (hp + 1) * DV)
        in_maps.append(
            {
                "condT": condT[b],
                "xT": xT[b],
                "Wq": np.ascontiguousarray(Wq[:, ds]),
                "Wk": np.ascontiguousarray(Wk[:, ds]),
                "Wv": np.ascontiguousarray(Wv[:, ds]),
                "Wu": np.ascontiguousarray(Wu[ds, :]),
            }
        )

    nc = _get_nc()
    res = run_bass_kernel_spmd(nc, in_maps, core_ids=list(range(NCORES)))
    results = res.results

    out = np.zeros((B, T, C), dtype=np.float32)
    for core in range(NCORES):
        out[core // 4] += np.asarray(results[core]["out"], dtype=np.float32)
    out += b_u
    return out
